# revision 22
# baseline (speedup 1.0000x reference)
"""Fused pre-LN transformer block (causal MHA + FFN) on 8 TRN2 NeuronCores.

Sharding: core c handles batch b = c//2 and head-half hh = c%2 (8 of 16 heads).
Attention runs fully local per (batch, head-half). The attention projection
partial sums are ReduceScattered (bf16, over the token dim) within each core
pair in TWO token-chunks so the collective overlaps attention/FFN compute.
FFN runs token-sharded (1024 tokens/core, 2 chunks of 512) fully in
feature-major layout (no transposes; the RS result is loaded back with
DMA-transpose). Output is written feature-major [E, TH]; the host transposes.

All matmuls run in bf16 (weights folded with LN gains and pre-cast on host),
accumulation in fp32 PSUM. The x' residual path stays fp32 on-chip.
"""

import numpy as np
import ml_dtypes

import concourse.bass as bass
import concourse.mybir as mybir
import concourse.tile as tile
from concourse import bacc
from concourse.bass import ts, ds
from concourse.bass_utils import run_bass_kernel_spmd

BF16 = mybir.dt.bfloat16
F32 = mybir.dt.float32
NPBF16 = ml_dtypes.bfloat16

B, T, E = 4, 2048, 1024
H, HS = 16, 64
FF = 4 * E
EPS = 1e-5
NCORES = 8
HPC = 8            # heads per core
HD = HPC * HS      # 512 head dims per core
TH = T // 2        # 1024 tokens per core for FFN
KT_N = T // 128    # 16 k-tiles
ET = E // 128      # 8 e-tiles
FJ = FF // 128     # 32 ff-tiles
ADD = mybir.AluOpType.add
SUB = mybir.AluOpType.subtract
MUL = mybir.AluOpType.mult


def _ln_rows(nc, pool, pspool, tiles, dt, ones, ones_row_f32, eps_sb, tagp,
             bc_via="pe"):
    """e-major LN over one 512-token chunk given as 8 tile APs [128, 512].

    Returns two PSUM tiles [128, 512] (partition-broadcast rstd and mu*rstd)
    so the normalize step is h = x * rstd_bc - ms_bc."""
    xsq = pool.tile([128, ET, 512], dt, tag=tagp + "xsq", bufs=1)
    for i in range(ET):
        nc.scalar.square(xsq[:, i, :], tiles[i])
    ps_sum = pspool.tile([1, 512], F32, tag=tagp + "st", bufs=2)
    ps_ssq = pspool.tile([1, 512], F32, tag=tagp + "st", bufs=2)
    for i in range(ET):
        nc.tensor.matmul(ps_sum, ones, tiles[i],
                         start=(i == 0), stop=(i == ET - 1))
    for i in range(ET):
        nc.tensor.matmul(ps_ssq, ones, xsq[:, i, :],
                         start=(i == 0), stop=(i == ET - 1))
    # Broadcast S and Q to all partitions FIRST (ACT row copy + PE ones
    # outer product), then do the row math as fast 128-partition DVE ops:
    # v = Q + E*eps - S^2/E ; rstd = 1/sqrt(v/E) ; ms = (S/E)*rstd
    s_row = pool.tile([1, 512], F32, tag=tagp + "sr", bufs=1)
    q_row = pool.tile([1, 512], F32, tag=tagp + "qr", bufs=1)
    nc.scalar.activation(s_row, ps_sum, mybir.ActivationFunctionType.Copy)
    nc.scalar.activation(q_row, ps_ssq, mybir.ActivationFunctionType.Copy)
    if bc_via == "pe":
        s_bc = pspool.tile([128, 512], F32, tag=tagp + "bc", bufs=2)
        q_bc = pspool.tile([128, 512], F32, tag=tagp + "bc", bufs=2)
        nc.tensor.matmul(s_bc, ones_row_f32, s_row, start=True, stop=True)
        nc.tensor.matmul(q_bc, ones_row_f32, q_row, start=True, stop=True)
    else:
        s_bc = pool.tile([128, 512], F32, tag=tagp + "bc", bufs=2)
        q_bc = pool.tile([128, 512], F32, tag=tagp + "bc", bufs=2)
        nc.gpsimd.partition_broadcast(s_bc, s_row)
        nc.gpsimd.partition_broadcast(q_bc, q_row)
    mu = pool.tile([128, 512], F32, tag=tagp + "mu", bufs=1)
    nc.vector.tensor_scalar_mul(mu, s_bc, 1.0 / E)
    u = pool.tile([128, 512], F32, tag=tagp + "u2", bufs=1)
    nc.vector.tensor_tensor(u, mu, s_bc, MUL)
    v = pool.tile([128, 512], F32, tag=tagp + "vr", bufs=1)
    nc.vector.scalar_tensor_tensor(v, q_bc, float(E) * EPS, u,
                                   op0=ADD, op1=SUB)
    nc.scalar.activation(v, v, mybir.ActivationFunctionType.Sqrt,
                         scale=1.0 / E)
    rstd_bc = pool.tile([128, 512], F32, tag=tagp + "rs", bufs=2)
    nc.vector.reciprocal(rstd_bc, v)
    ms_bc = pool.tile([128, 512], F32, tag=tagp + "ms", bufs=2)
    nc.vector.tensor_tensor(ms_bc, mu, rstd_bc, MUL)
    return rstd_bc, ms_bc


def build_program(single=False):
    nc = bacc.Bacc("TRN2", target_bir_lowering=False, debug=False,
                   num_devices=1 if single else NCORES)

    # ---- I/O (host pre-tiled for contiguous per-partition DMA) ----
    xtp = nc.dram_tensor("xtp", [4, 128, ET, 512], BF16,
                         kind="ExternalInput").ap()    # [c,p,i,t] = x.T chunks
    xrt = nc.dram_tensor("xrt", [2, 128, ET, 512], BF16,
                         kind="ExternalInput").ap()    # (x.T+apb)[:, my half]
    qws_d = nc.dram_tensor("qws", [128, ET, HD], BF16,
                           kind="ExternalInput").ap()
    kws_d = nc.dram_tensor("kws", [128, ET, HD], BF16,
                           kind="ExternalInput").ap()
    vws_d = nc.dram_tensor("vws", [128, ET, HD], BF16,
                           kind="ExternalInput").ap()
    apws_d = nc.dram_tensor("apws", [128, 4, E], BF16,
                            kind="ExternalInput").ap()
    fw1t = nc.dram_tensor("fw1t", [FJ, 128, ET, 128], BF16,
                          kind="ExternalInput").ap()
    fb1 = nc.dram_tensor("fb1", [128, FJ], F32, kind="ExternalInput").ap()
    fw2 = nc.dram_tensor("fw2", [FF, E], BF16, kind="ExternalInput").ap()
    fb2t = nc.dram_tensor("fb2t", [128, ET], F32, kind="ExternalInput").ap()
    outT = nc.dram_tensor("outT", [E, TH], F32, kind="ExternalOutput").ap()

    # internal DRAM for the pair-wise chunked reduce-scatter
    part = [nc.dram_tensor(f"part{k}", [2048, 512], BF16).ap()
            for k in range(2)]
    rs = [nc.dram_tensor(f"rs{k}", [1024, 512], BF16).ap()
          for k in range(2)]
    groups = [[0, 1], [2, 3], [4, 5], [6, 7]]

    with tile.TileContext(nc) as tc:
        with tc.tile_pool(name="const", bufs=1) as constp:
            ones_bf = constp.tile([128, 1], BF16)
            nc.gpsimd.memset(ones_bf, 1.0)
            ones_row_f32 = constp.tile([1, 128], F32)
            nc.gpsimd.memset(ones_row_f32, 1.0)
            ones_f32 = constp.tile([128, 1], F32)
            nc.gpsimd.memset(ones_f32, 1.0)
            # causal mask tile: mask[k, q] = 1 if q >= k else 0
            mask = constp.tile([128, 128], BF16)
            nc.gpsimd.memset(mask, 1.0)
            nc.gpsimd.affine_select(
                out=mask, in_=mask, compare_op=mybir.AluOpType.is_ge,
                fill=0.0, base=0, pattern=[[1, 128]], channel_multiplier=-1)
            fb1_sb = constp.tile([128, FJ], F32)
            nc.sync.dma_start(out=fb1_sb, in_=fb1)
            fb2_sb = constp.tile([128, ET], F32)
            nc.sync.dma_start(out=fb2_sb, in_=fb2t)
            eps_sb = constp.tile([128, 1], F32)
            nc.gpsimd.memset(eps_sb, EPS)
            # x' (post-attention residual), fp32, lives to the end
            xp = constp.tile([128, 2, ET, 512], F32, name="xp")
            xrt_sb = constp.tile([128, 2, ET, 512], BF16, name="xrt_sb")

            with tc.tile_pool(name="persA", bufs=1) as pA:
                QT = pA.tile([128, 4, T], BF16, name="QT")
                KT = pA.tile([128, 4, T], BF16, name="KT")
                Vp = pA.tile([128, KT_N, HPC, 65], BF16, name="Vp")
                AO = pA.tile([128, 4, T], BF16, name="AO")
                apws = pA.tile([128, 4, E], BF16, name="apws")

                # ======== phase A1: LN1 + QKV (per 512-token chunk) ========
                with tc.tile_pool(name="ln1", bufs=1) as sb, \
                     tc.tile_pool(name="ln1_ps", bufs=1, space="PSUM") as ps:
                    qws = sb.tile([128, ET, HD], BF16, name="qws")
                    kws = sb.tile([128, ET, HD], BF16, name="kws")
                    vws = sb.tile([128, ET, HD], BF16, name="vws")
                    xTc0 = sb.tile([128, ET, 512], BF16, tag="xTc",
                                   bufs=2)
                    nc.sync.dma_start(out=xTc0, in_=xtp[0])
                    nc.sync.dma_start(out=qws, in_=qws_d)
                    nc.sync.dma_start(out=kws, in_=kws_d)
                    xTc1 = sb.tile([128, ET, 512], BF16, tag="xTc", bufs=2)
                    nc.sync.dma_start(out=xTc1, in_=xtp[1])
                    nc.sync.dma_start(out=vws, in_=vws_d)
                    nc.sync.dma_start(out=xrt_sb[:, 0], in_=xrt[0])
                    nc.sync.dma_start(out=xrt_sb[:, 1], in_=xrt[1])
                    nc.sync.dma_start(out=apws, in_=apws_d)

                    for c in range(4):
                        sl = ds(512 * c, 512)
                        if c < 2:
                            xTc = (xTc0, xTc1)[c]
                        else:
                            xTc = sb.tile([128, ET, 512], BF16, tag="xTc",
                                          bufs=2)
                            nc.sync.dma_start(out=xTc, in_=xtp[c])
                        rstd_bc, ms_bc = _ln_rows(
                            nc, sb, ps, [xTc[:, i, :] for i in range(ET)],
                            BF16, ones_bf, ones_row_f32, eps_sb, "a")
                        hTc = sb.tile([128, ET, 512], BF16, tag="hTc", bufs=2)
                        for i in range(ET):
                            t0 = sb.tile([128, 512], F32, tag="lnt", bufs=2)
                            nc.vector.tensor_tensor(t0, xTc[:, i, :],
                                                    rstd_bc, MUL)
                            nc.vector.tensor_tensor(hTc[:, i, :], t0,
                                                    ms_bc, SUB)
                        # QT/KT for this chunk
                        for w_sb, o_sb in ((qws, QT), (kws, KT)):
                            for fi in range(4):
                                pq = ps.tile([128, 512], F32, tag="qkv",
                                             bufs=4)
                                for i in range(ET):
                                    nc.tensor.matmul(
                                        pq, w_sb[:, i, ts(fi, 128)],
                                        hTc[:, i, :],
                                        start=(i == 0), stop=(i == ET - 1))
                                nc.vector.tensor_copy(o_sb[:, fi, sl], pq)
                        # V (token-major, with ones column) for this chunk
                        for t4 in range(4):
                            ti = 4 * c + t4
                            pv = ps.tile([128, 512], F32, tag="qkv", bufs=4)
                            for i in range(ET):
                                nc.tensor.matmul(pv, hTc[:, i, ts(t4, 128)],
                                                 vws[:, i, :],
                                                 start=(i == 0),
                                                 stop=(i == ET - 1))
                            nc.vector.tensor_copy(
                                Vp[:, ti, :, 0:64],
                                pv.rearrange("p (h d) -> p h d", h=HPC))
                            nc.vector.memset(Vp[:, ti, :, 64:65], 1.0)

                # ======== phase A2+A3: attention, proj, chunked RS ========
                with tc.tile_pool(name="att", bufs=1) as sb, \
                     tc.tile_pool(name="att_ps", bufs=1, space="PSUM") as ps:

                    def attend(c):
                        jmax = 4 * c + 3
                        for hp in range(4):
                            psO = [ps.tile([65, 512], F32, tag=f"o{z}",
                                           bufs=2, name=f"psO{z}")
                                   for z in range(2)]
                            for j in range(jmax + 1):
                                d0 = max(0, 128 * j - 512 * c)
                                n = 512 - d0
                                diag = (j // 4 == c)
                                pS = [ps.tile([128, 512], F32, tag=f"s{z}",
                                              bufs=2, name=f"pS{z}")
                                      for z in range(2)]
                                PTt = [sb.tile([128, 512], BF16,
                                               tag=f"pt{z}", bufs=3,
                                               name=f"PTt{z}")
                                       for z in range(2)]
                                # the two z-score matmuls use disjoint
                                # 64-row groups -> issue adjacently so the
                                # PE runs them concurrently
                                for z in range(2):
                                    pp = slice(64 * z, 64 * z + 64)
                                    nc.tensor.matmul(
                                        pS[z][:, d0:512],
                                        KT[pp, hp, ts(j, 128)],
                                        QT[pp, hp, ds(512 * c + d0, n)],
                                        start=True, stop=True)
                                for z in range(2):
                                    nc.scalar.activation(
                                        PTt[z][:, d0:512], pS[z][:, d0:512],
                                        mybir.ActivationFunctionType.Exp,
                                        scale=float(HS) ** -0.5)
                                    if d0 > 0:
                                        nc.vector.memset(PTt[z][:, 0:d0], 0.0)
                                    if diag:
                                        nc.vector.tensor_tensor(
                                            PTt[z][:, d0:d0 + 128],
                                            PTt[z][:, d0:d0 + 128], mask, MUL)
                                for z in range(2):
                                    nc.tensor.matmul(
                                        psO[z], Vp[:, j, 2 * hp + z, :],
                                        PTt[z], start=(j == 0),
                                        stop=(j == jmax))
                            # normalize: O[d, q] / l[q], l = psum row 64
                            for z in range(2):
                                rl = sb.tile([1, 512], F32, tag="rl", bufs=3)
                                nc.scalar.activation(
                                    rl, psO[z][64:65, :],
                                    mybir.ActivationFunctionType.Copy)
                                lb = sb.tile([64, 512], F32, tag="lb",
                                             bufs=3)
                                nc.gpsimd.partition_broadcast(lb, rl)
                                rlb = sb.tile([64, 512], F32, tag="rlb",
                                              bufs=3)
                                nc.vector.reciprocal(rlb, lb)
                                nc.vector.tensor_tensor(
                                    AO[slice(64 * z, 64 * z + 64), hp,
                                       ds(512 * c, 512)],
                                    psO[z][0:64, :], rlb, MUL)

                    def proj_rs(k):
                        # RS chunk k covers global 512-blocks k (rank0 rows,
                        # feature-major [E, 512]) and k+2 (rank1 rows)
                        for half in range(2):
                            g = k + 2 * half
                            for et in range(ET):
                                pp = ps.tile([128, 512], F32,
                                             tag=f"s{et % 2}", bufs=2)
                                for hi in range(4):
                                    nc.tensor.matmul(
                                        pp, apws[:, hi, ts(et, 128)],
                                        AO[:, hi, ds(512 * g, 512)],
                                        start=(hi == 0), stop=(hi == 3))
                                po = sb.tile([128, 512], BF16, tag="postage",
                                             bufs=4)
                                nc.vector.tensor_copy(po, pp)
                                nc.sync.dma_start(
                                    out=part[k][ds(1024 * half + 128 * et,
                                                   128), :],
                                    in_=po)
                        if single:
                            nc.sync.dma_start(out=rs[k],
                                              in_=part[k][0:1024, :])
                        else:
                            nc.gpsimd.collective_compute(
                                "ReduceScatter", ADD,
                                replica_groups=groups,
                                ins=[part[k][:]], outs=[rs[k][:]])

                    attend(0)
                    attend(2)
                    proj_rs(0)
                    attend(1)
                    attend(3)
                    proj_rs(1)

            # ======== phase B: LN2 + FFN per 512-token chunk ========
            with tc.tile_pool(name="ffn", bufs=1) as sb, \
                 tc.tile_pool(name="ffn_ps", bufs=1, space="PSUM") as ps:
                for k in range(2):
                    tsl = ds(512 * k, 512)
                    if k == 0:
                        # x'^T(0) = rs^T (already feature-major) + (x^T+apb)
                        for i in range(ET):
                            rst = sb.tile([128, 512], BF16, tag="rst", bufs=3)
                            nc.scalar.dma_start(out=rst,
                                                in_=rs[0][ts(i, 128), :])
                            nc.vector.tensor_tensor(xp[:, 0, i, :], rst,
                                                    xrt_sb[:, 0, i, :], ADD)
                    rstd_bc, ms_bc = _ln_rows(
                        nc, sb, ps, [xp[:, k, i, :] for i in range(ET)],
                        F32, ones_f32, ones_row_f32, eps_sb, "b")
                    h2T = sb.tile([128, ET, 512], BF16, tag="h2T", bufs=2)
                    for i in range(ET):
                        t0 = sb.tile([128, 512], F32, tag="lnt2", bufs=3)
                        nc.vector.tensor_tensor(t0, xp[:, k, i, :],
                                                rstd_bc, MUL)
                        nc.vector.tensor_tensor(h2T[:, i, :], t0, ms_bc, SUB)
                    # ff1: ffh[fj, tok] = relu(W1^T @ h2 + b1)
                    ffh = sb.tile([128, FJ, 512], BF16, tag="ffh", bufs=1)
                    for fj in range(FJ):
                        w1 = sb.tile([128, ET, 128], BF16, tag="w1t", bufs=4)
                        nc.sync.dma_start(out=w1, in_=fw1t[fj])
                        pf = ps.tile([128, 512], F32, tag="bst", bufs=2)
                        for i in range(ET):
                            nc.tensor.matmul(pf, w1[:, i, :], h2T[:, i, :],
                                             start=(i == 0),
                                             stop=(i == ET - 1))
                        nc.scalar.activation(
                            ffh[:, fj, :], pf,
                            mybir.ActivationFunctionType.Relu,
                            bias=fb1_sb[:, fj:fj + 1])
                    if k == 0:
                        # x'(k1) assembly: rs[1] lands mid-FFN(k0).  Keep the
                        # whole chain on the (otherwise idle) gpsimd queue so
                        # a slow collective cannot block the sync/DVE queues.
                        for i in range(ET):
                            rst = sb.tile([128, 512], BF16, tag="rst2",
                                          bufs=3)
                            nc.gpsimd.dma_start(out=rst,
                                                in_=rs[1][ts(i, 128), :])
                            nc.gpsimd.tensor_tensor(xp[:, 1, i, :], rst,
                                                    xrt_sb[:, 1, i, :], ADD)
                    # ff2 in two E-halves: 4 psum accumulators each
                    for eh in range(2):
                        pg = [ps.tile([128, 512], F32, tag="pg", bufs=4,
                                      name=f"pg{et}") for et in range(4)]
                        for fj in range(FJ):
                            w2 = sb.tile([128, 512], BF16, tag="w2t", bufs=4)
                            nc.sync.dma_start(
                                out=w2,
                                in_=fw2[ts(fj, 128), ds(512 * eh, 512)])
                            for et in range(4):
                                nc.tensor.matmul(
                                    pg[et], w2[:, ts(et, 128)],
                                    ffh[:, fj, :],
                                    start=(fj == 0), stop=(fj == FJ - 1))
                        for et in range(4):
                            eg = 4 * eh + et
                            f1 = sb.tile([128, 512], F32, tag="f1", bufs=4)
                            nc.vector.scalar_tensor_tensor(
                                f1, pg[et], fb2_sb[:, eg:eg + 1],
                                xp[:, k, eg, :], op0=ADD, op1=ADD)
                            nc.sync.dma_start(out=outT[ts(eg, 128), tsl],
                                              in_=f1)

    nc.compile()
    return nc


_CACHED = {}


def _prepare_inputs(x, qkv_w, attn_proj_w, attn_proj_b, ln1_g, ln1_b,
                    ln2_g, ln2_b, ff_w1, ff_b1, ff_w2, ff_b2):
    """Fold LN affine params into the weights, shard, pre-tile, cast bf16."""
    x = np.asarray(x, np.float32)
    qkv_w = np.asarray(qkv_w, np.float32) * np.asarray(ln1_g, np.float32)[:, None]
    qkv_b = np.asarray(ln1_b, np.float32) @ qkv_w  # [3*H*HS]
    assert np.abs(qkv_b).max() == 0.0, "nonzero ln1_b not supported"
    ff_w1f = np.asarray(ff_w1, np.float32) * np.asarray(ln2_g, np.float32)[:, None]
    ff_b1f = np.asarray(ff_b1, np.float32) + np.asarray(ln2_b, np.float32) @ ff_w1f
    apb = np.asarray(attn_proj_b, np.float32)

    def tile_w(w, nt):  # [E_in, F] -> [128, nt, F] with row = i*128+p
        return np.ascontiguousarray(
            w.reshape(nt, 128, w.shape[1]).transpose(1, 0, 2)).astype(NPBF16)

    fw1t = np.ascontiguousarray(
        ff_w1f.reshape(ET, 128, FJ, 128).transpose(2, 1, 0, 3)).astype(NPBF16)
    fb1_t = np.ascontiguousarray(ff_b1f.reshape(FJ, 128).T)
    fw2_bf = np.asarray(ff_w2, np.float32).astype(NPBF16)
    fb2_t = np.ascontiguousarray(
        np.asarray(ff_b2, np.float32).reshape(ET, 128).T)
    apw = np.asarray(attn_proj_w, np.float32)

    in_maps = []
    for c in range(NCORES):
        b, hh = c // 2, c % 2
        hsl = slice(512 * hh, 512 * hh + 512)
        xT = x[b].T  # [E, T]
        xtp = np.ascontiguousarray(
            xT.reshape(ET, 128, 4, 512).transpose(2, 1, 0, 3)).astype(NPBF16)
        xrt = np.ascontiguousarray(
            (xT + apb[:, None])[:, TH * hh:TH * hh + TH]
            .reshape(ET, 128, 2, 512).transpose(2, 1, 0, 3)).astype(NPBF16)
        in_maps.append({
            "xtp": xtp,
            "xrt": xrt,
            "qws": tile_w(np.ascontiguousarray(qkv_w[:, hsl]), ET),
            "kws": tile_w(np.ascontiguousarray(qkv_w[:, H * HS:][:, hsl]), ET),
            "vws": tile_w(np.ascontiguousarray(qkv_w[:, 2 * H * HS:][:, hsl]),
                          ET),
            "apws": tile_w(np.ascontiguousarray(apw[hsl, :]), 4),
            "fw1t": fw1t,
            "fb1": fb1_t,
            "fw2": fw2_bf,
            "fb2t": fb2_t,
        })
    return in_maps


def _assemble(fetch):
    full = np.empty((B, T, E), np.float32)
    for c in range(NCORES):
        b, hh = c // 2, c % 2
        full[b, TH * hh:TH * hh + TH] = fetch(c, "outT").T
    return full


def kernel(**inputs):
    if "nc" not in _CACHED:
        _CACHED["nc"] = build_program()
    nc = _CACHED["nc"]
    in_maps = _prepare_inputs(**inputs)
    res = run_bass_kernel_spmd(nc, in_maps, list(range(NCORES)))
    return _assemble(lambda c, n: res.results[c][n])
